# revision 9
# baseline (speedup 1.0000x reference)
"""Trainium2 Bass kernel for GQA attention with alibi + RoPE + causal mask.

Problem: B=2, S=2048, DIM=2048, 32 q-heads, 8 kv-heads, head_dim=64.
out = softmax(rope(xWq) rope(xWk)^T / 8 + alibi + causal) (xWv) @ Wo

Sharding (8 cores): core g owns kv-head g and q-heads 4g..4g+3, both
batches (alibi tiles are reused across the batch dim). Wo is split by
input dim; each core emits a partial [4096, 2048] output which the host
sums.

Device dataflow is fully "transposed" so no on-chip transposes of big
tensors are needed:
  - host passes xT [DIM, B*S]; projections compute QT/KT/VT = W^T xT
    directly via PE matmuls (weights stationary). Wq is split into
    even-d / odd-d column planes so RoPE runs as full-width aligned
    DVE ops; planes are then merged into head-contiguous layout via
    SBUF->SBUF DMAs (compute engines require same-partition operands,
    DMAs don't). 1/sqrt(hd) is folded into Wq on the host.
  - scores are built as S^T[k, q] tiles; alibi (host-transposed, causal
    mask folded in as -1e9) is accumulated into the same PSUM tile with
    an identity matmul; exp runs on the scalar engine (no max-subtraction
    needed: scores are O(10)); PV contracts k with V' = [V | 1] stationary
    so row 64 of the PSUM accumulator is the softmax denominator.
  - out^T[d, q] tiles are exactly the lhsT the Wo projection needs.

Matmuls run in float32r (full-rate fp32 PE mode).
"""

import os

import numpy as np
import ml_dtypes

BF16 = ml_dtypes.bfloat16
B = 2
S = 2048
DIM = 2048
N_HEADS = 32
N_KV_HEADS = 8
HEAD_DIM = 64
HL = 4            # local q heads per core
N_CORES = 8
BS = B * S        # 4096
KT = DIM // 128   # 16 contraction tiles for projections
MS = 512          # m-strip width for projections
NSTRIP = BS // MS  # 8
QS = 512          # q-strip width in attention
ROPE_THETA = 10000.0
NEG = -1e9

_compiled = None


def _build_nc(iters=1):
    import concourse.bass as bass
    import concourse.tile as tile
    from concourse import bacc, mybir
    from concourse.masks import make_identity

    f32 = mybir.dt.float32
    bf16 = mybir.dt.bfloat16
    SUB = mybir.AluOpType.subtract
    ADD = mybir.AluOpType.add

    nc = bacc.Bacc("TRN2", target_bir_lowering=False, debug=False,
                   num_devices=N_CORES)

    xT = nc.dram_tensor("xT", [DIM, BS], bf16, kind="ExternalInput").ap()
    wqe = nc.dram_tensor("wqe", [DIM, 128], bf16, kind="ExternalInput").ap()
    wqo = nc.dram_tensor("wqo", [DIM, 128], bf16, kind="ExternalInput").ap()
    wkv = nc.dram_tensor("wkv", [DIM, 128], bf16, kind="ExternalInput").ap()
    wo = nc.dram_tensor("wo", [HL * HEAD_DIM, DIM], bf16, kind="ExternalInput").ap()
    alibiT = nc.dram_tensor("alibiT", [HL, S, S], bf16, kind="ExternalInput").ap()
    c128 = nc.dram_tensor("c128", [128, S], f32, kind="ExternalInput").ap()
    s128 = nc.dram_tensor("s128", [128, S], f32, kind="ExternalInput").ap()
    out = nc.dram_tensor("out", [BS, DIM], f32, kind="ExternalOutput").ap()

    with tile.TileContext(nc) as tc, \
         tc.tile_pool(name="persist", bufs=1) as pers:
        # ---- persistent SBUF ----
        wqe_sb = pers.tile([128, KT, 128], bf16, name="wqe_sb")
        wqo_sb = pers.tile([128, KT, 128], bf16, name="wqo_sb")
        wkv_sb = pers.tile([128, KT, 128], bf16, name="wkv_sb")
        wo_sb = pers.tile([128, 2, DIM], bf16, name="wo_sb")
        c128_sb = pers.tile([128, S], f32, name="c128_sb")
        s128_sb = pers.tile([128, S], f32, name="s128_sb")
        ident = pers.tile([128, 128], bf16, name="ident")
        ones65 = pers.tile([65, 64], f32, name="ones65")
        qt0 = pers.tile([128, BS], bf16, name="qt0")   # heads 0,1 (rope'd QT)
        qt1 = pers.tile([128, BS], bf16, name="qt1")   # heads 2,3
        kt_sb = pers.tile([128, BS], bf16, name="kt_sb")  # KT dup'd both halves
        v_sb = pers.tile([128, 32, 65], bf16, name="v_sb")   # V' blocks [V|1]
        ot0 = pers.tile([128, BS], bf16, name="ot0")   # heads 0,1 outT
        ot1 = pers.tile([128, BS], bf16, name="ot1")

        nc.sync.dma_start(wqe_sb[:], wqe.rearrange("(t p) n -> p t n", p=128))
        nc.sync.dma_start(wqo_sb[:], wqo.rearrange("(t p) n -> p t n", p=128))
        nc.sync.dma_start(wkv_sb[:], wkv.rearrange("(t p) n -> p t n", p=128))
        nc.sync.dma_start(wo_sb[:], wo.rearrange("(t p) n -> p t n", p=128))
        nc.sync.dma_start(c128_sb[:], c128[:])
        nc.sync.dma_start(s128_sb[:], s128[:])
        make_identity(nc, ident[:])
        nc.vector.memset(ones65[:], 1.0)
        nc.vector.memset(v_sb[:, :, 64:65], 1.0)

        for _ in range(iters):
            # ================= phase 1: projections =================
            with tc.tile_pool(name="proj_x", bufs=2) as xp, \
                 tc.tile_pool(name="proj_ps", bufs=2, space="PSUM") as pp, \
                 tc.tile_pool(name="proj_tmp", bufs=8) as tp:
                for st in range(NSTRIP):
                    m0 = st * MS
                    s0 = m0 % S
                    qpe = pp.tile([128, MS], f32, tag="qpe", name="qpe")
                    qpo = pp.tile([128, MS], f32, tag="qpo", name="qpo")
                    kvp = pp.tile([128, MS], f32, tag="kvp", name="kvp")
                    for tq in range(4):   # quarters of the k loop
                        xt = xp.tile([128, 4, MS], bf16, tag="xt", name="xt")
                        nc.sync.dma_start(
                            xt[:],
                            xT[tq * 512:(tq + 1) * 512, m0:m0 + MS]
                            .rearrange("(t p) m -> p t m", p=128))
                        for t4 in range(4):
                            t = tq * 4 + t4
                            first = t == 0
                            last = t == KT - 1
                            for wsb, ps in ((wqe_sb, qpe), (wqo_sb, qpo),
                                            (wkv_sb, kvp)):
                                nc.tensor.matmul(
                                    ps[:], wsb[:, t, :],
                                    xt[:, t4, :],
                                    start=first, stop=last)
                    # ---- RoPE Q: full-width plane ops (all 4 heads) ----
                    qe_r = tp.tile([128, MS], bf16, tag="tmp", name="qe_r")
                    qo_r = tp.tile([128, MS], bf16, tag="tmp", name="qo_r")
                    t1 = tp.tile([128, MS], f32, tag="tmp", name="t1")
                    t2 = tp.tile([128, MS], f32, tag="tmp", name="t2")
                    cS = c128_sb[:, s0:s0 + MS]
                    sS = s128_sb[:, s0:s0 + MS]
                    nc.vector.tensor_mul(t1[:], qpe[:], cS)
                    nc.vector.tensor_mul(t2[:], qpo[:], sS)
                    nc.vector.tensor_tensor(qe_r[:], t1[:], t2[:], op=SUB)
                    t3 = tp.tile([128, MS], f32, tag="tmp", name="t3")
                    t4_ = tp.tile([128, MS], f32, tag="tmp", name="t4_")
                    nc.vector.tensor_mul(t3[:], qpe[:], sS)
                    nc.vector.tensor_mul(t4_[:], qpo[:], cS)
                    nc.vector.tensor_tensor(qo_r[:], t3[:], t4_[:], op=ADD)
                    # merge planes -> head-contiguous qt banks (DMA moves
                    # partitions; compute engines can't)
                    for h in range(HL):
                        qtb = (qt0, qt1)[h // 2]
                        hb = (h % 2) * 64
                        nc.sync.dma_start(
                            qtb[hb:hb + 32, m0:m0 + MS],
                            qe_r[h * 32:(h + 1) * 32, :])
                        nc.sync.dma_start(
                            qtb[hb + 32:hb + 64, m0:m0 + MS],
                            qo_r[h * 32:(h + 1) * 32, :])
                    # ---- RoPE K (rows 0-63 of kvp) ----
                    km = tp.tile([64, MS], f32, tag="tmp", name="km")
                    ka = tp.tile([32, MS], f32, tag="tmp", name="ka")
                    ko2 = tp.tile([32, MS], bf16, tag="tmp", name="ko2")
                    nc.vector.tensor_mul(km[0:32, :], kvp[0:32, :],
                                         c128_sb[0:32, s0:s0 + MS])
                    nc.vector.tensor_mul(km[32:64, :], kvp[32:64, :],
                                         s128_sb[32:64, s0:s0 + MS])
                    nc.sync.dma_start(ka[:], km[32:64, :])
                    nc.vector.tensor_tensor(kt_sb[0:32, m0:m0 + MS],
                                            km[0:32, :], ka[:], op=SUB)
                    km2 = tp.tile([64, MS], f32, tag="tmp", name="km2")
                    ka2 = tp.tile([32, MS], f32, tag="tmp", name="ka2")
                    nc.vector.tensor_mul(km2[0:32, :], kvp[0:32, :],
                                         s128_sb[0:32, s0:s0 + MS])
                    nc.vector.tensor_mul(km2[32:64, :], kvp[32:64, :],
                                         c128_sb[32:64, s0:s0 + MS])
                    nc.sync.dma_start(ka2[:], km2[32:64, :])
                    nc.vector.tensor_tensor(ko2[:], km2[0:32, :], ka2[:], op=ADD)
                    nc.sync.dma_start(kt_sb[32:64, m0:m0 + MS], ko2[:])
                    # duplicate rope'd KT into partitions 64-127 (matmul
                    # needs lhsT and rhs at the same base partition)
                    nc.sync.dma_start(kt_sb[64:128, m0:m0 + MS],
                                      kt_sb[0:64, m0:m0 + MS])
                    # ---- V (rows 64-127 of kvp): transpose to [k, d] ----
                    vts = tp.tile([128, MS], bf16, tag="tmp", name="vts")
                    nc.any.tensor_copy(vts[64:128, :], kvp[64:128, :])
                    for c in range(4):
                        vtp = pp.tile([128, 64], bf16, tag="vtp", name="vtp")
                        nc.tensor.transpose(vtp[:],
                                            vts[64:128, c * 128:(c + 1) * 128],
                                            ident[64:128, 64:128])
                        nc.any.tensor_copy(v_sb[:, st * 4 + c, 0:64], vtp[:])

            # ================= phase 2: attention =================
            with tc.tile_pool(name="att_al", bufs=4) as alp, \
                 tc.tile_pool(name="att_es", bufs=4) as esp, \
                 tc.tile_pool(name="att_st", bufs=3, space="PSUM") as stp, \
                 tc.tile_pool(name="att_ov", bufs=2, space="PSUM") as ovp, \
                 tc.tile_pool(name="att_rb", bufs=1, space="PSUM") as rbp, \
                 tc.tile_pool(name="att_sm", bufs=2) as smp:
                for h in range(HL):
                    qtb = (qt0, qt1)[h // 2]
                    otb = (ot0, ot1)[h // 2]
                    hb = (h % 2) * 64
                    for qs in range(S // QS):
                        q0 = qs * QS
                        nk = 4 * (qs + 1)
                        ov = [ovp.tile([65, QS], f32, tag=f"ov{b}", name=f"ov{b}")
                              for b in range(B)]
                        for jk in range(nk):
                            k0 = jk * 128
                            al = alp.tile([128, QS], bf16, tag="al", name="al")
                            nc.sync.dma_start(
                                al[:], alibiT[h, k0:k0 + 128, q0:q0 + QS])
                            for b in range(B):
                                stt = stp.tile([128, QS], f32, tag="st", name="st")
                                nc.tensor.matmul(
                                    stt[:],
                                    kt_sb[hb:hb + 64,
                                          b * S + k0:b * S + k0 + 128]
                                    ,
                                    qtb[hb:hb + 64, b * S + q0:b * S + q0 + QS]
                                    ,
                                    start=True, stop=False)
                                nc.tensor.matmul(
                                    stt[:], ident[:],
                                    al[:],
                                    start=False, stop=True)
                                es = esp.tile([128, QS], bf16, tag="es", name="es")
                                nc.scalar.activation(
                                    es[:], stt[:],
                                    mybir.ActivationFunctionType.Exp)
                                nc.tensor.matmul(
                                    ov[b][:],
                                    v_sb[:, b * 16 + jk, :],
                                    es[:],
                                    start=(jk == 0), stop=(jk == nk - 1))
                        for b in range(B):
                            rc = smp.tile([65, QS], f32, tag="rc", name="rc")
                            nc.vector.reciprocal(rc[64:65, :], ov[b][64:65, :])
                            rb = rbp.tile([64, QS], f32, tag="rb", name="rb")
                            nc.tensor.matmul(
                                rb[:], ones65[64:65, :],
                                rc[64:65, :],
                                start=True, stop=True)
                            rbs = smp.tile([64, QS], f32, tag="rbs",
                                           name="rbs")
                            nc.any.tensor_copy(rbs[:], rb[:])
                            dst = b * S + q0
                            if hb == 0:
                                nc.vector.tensor_mul(
                                    otb[0:64, dst:dst + QS],
                                    ov[b][0:64, :], rbs[:])
                            else:
                                om = smp.tile([64, QS], bf16, tag="om", name="om")
                                nc.vector.tensor_mul(om[:], ov[b][0:64, :],
                                                     rbs[:])
                                nc.sync.dma_start(otb[64:128, dst:dst + QS],
                                                  om[:])

            # ================= phase 3: output projection =================
            with tc.tile_pool(name="out_ps", bufs=3, space="PSUM") as opp, \
                 tc.tile_pool(name="out_sb", bufs=3) as osp:
                for mt in range(BS // 128):
                    for ns in range(DIM // 512):
                        op = opp.tile([128, 512], f32, tag="op", name="op")
                        nc.tensor.matmul(
                            op[:], ot0[:, mt * 128:(mt + 1) * 128],
                            wo_sb[:, 0, ns * 512:(ns + 1) * 512],
                            start=True, stop=False)
                        nc.tensor.matmul(
                            op[:], ot1[:, mt * 128:(mt + 1) * 128],
                            wo_sb[:, 1, ns * 512:(ns + 1) * 512],
                            start=False, stop=True)
                        ob = osp.tile([128, 512], f32, tag="ob", name="ob")
                        nc.any.tensor_copy(ob[:], op[:])
                        nc.sync.dma_start(
                            out[mt * 128:(mt + 1) * 128,
                                ns * 512:(ns + 1) * 512], ob[:])

    nc.compile()
    return nc


def _rope_tables():
    inv = 1.0 / (ROPE_THETA ** (np.arange(0, HEAD_DIM, 2, dtype=np.float64)
                                / HEAD_DIM))
    ang = np.arange(S, dtype=np.float64)[None, :] * inv[:, None]   # [32, S]
    cos = np.cos(ang).astype(np.float32)
    sin = np.sin(ang).astype(np.float32)
    c128 = np.ascontiguousarray(np.tile(cos, (4, 1)))
    s128 = np.ascontiguousarray(np.tile(sin, (4, 1)))
    return c128, s128


def shard_inputs(x, alibi_bias, Wq, Wk, Wv, Wo):
    x = np.asarray(x, dtype=np.float32)
    alibi_bias = np.asarray(alibi_bias, dtype=np.float32)
    Wq = np.asarray(Wq, dtype=np.float32)
    Wk = np.asarray(Wk, dtype=np.float32)
    Wv = np.asarray(Wv, dtype=np.float32)
    Wo = np.asarray(Wo, dtype=np.float32)

    xT_bf = np.ascontiguousarray(x.reshape(BS, DIM).T).astype(BF16)
    c128, s128 = _rope_tables()
    ev = np.arange(0, HEAD_DIM, 2)
    od = np.arange(1, HEAD_DIM, 2)
    perm = np.concatenate([ev, od])
    # causal mask in transposed [k, q] layout: invalid where k > q
    ktri = np.tril(np.full((S, S), NEG, dtype=np.float32), k=-1)

    in_maps = []
    scale = np.float32(1.0 / np.sqrt(HEAD_DIM))
    for g in range(N_CORES):
        hs = np.arange(4 * g, 4 * g + 4)
        e_cols = np.concatenate([h * HEAD_DIM + ev for h in hs])
        o_cols = np.concatenate([h * HEAD_DIM + od for h in hs])
        wqe_g = np.ascontiguousarray(Wq[:, e_cols]) * scale
        wqo_g = np.ascontiguousarray(Wq[:, o_cols]) * scale
        wk_g = Wk[:, g * HEAD_DIM + perm]
        wv_g = Wv[:, g * HEAD_DIM:(g + 1) * HEAD_DIM]
        wkv_g = np.ascontiguousarray(np.concatenate([wk_g, wv_g], axis=1))
        wo_g = np.ascontiguousarray(Wo[4 * g * HEAD_DIM:(4 * g + 4) * HEAD_DIM])
        at = alibi_bias[0, hs].transpose(0, 2, 1) + ktri[None]
        in_maps.append({
            "xT": xT_bf,
            "wqe": wqe_g.astype(BF16),
            "wqo": wqo_g.astype(BF16),
            "wkv": wkv_g.astype(BF16),
            "wo": wo_g.astype(BF16),
            "alibiT": np.ascontiguousarray(at).astype(BF16),
            "c128": c128, "s128": s128,
        })
    return in_maps


def get_compiled(iters=None):
    global _compiled
    if iters is None:
        iters = int(os.environ.get("BASS_ITERS", "1"))
    if _compiled is None or _compiled[0] != iters:
        _compiled = (iters, _build_nc(iters))
    return _compiled[1]


def kernel(**inputs):
    from concourse.bass_utils import run_bass_kernel_spmd

    nc = get_compiled()
    in_maps = shard_inputs(**inputs)
    res = run_bass_kernel_spmd(nc, in_maps, core_ids=list(range(N_CORES)))
    total = np.zeros((BS, DIM), dtype=np.float32)
    for r in res.results:
        total += r["out"]
    return total.reshape(B, S, DIM)


# revision 10
# speedup vs baseline: 1.0062x; 1.0062x over previous
"""Trainium2 Bass kernel for GQA attention with alibi + RoPE + causal mask.

Problem: B=2, S=2048, DIM=2048, 32 q-heads, 8 kv-heads, head_dim=64.
out = softmax(rope(xWq) rope(xWk)^T / 8 + alibi + causal) (xWv) @ Wo

Sharding (8 cores): core g owns kv-head g and q-heads 4g..4g+3, both
batches (alibi tiles are reused across the batch dim). Wo is split by
input dim; each core emits a partial [4096, 2048] output which the host
sums.

Device dataflow is fully "transposed" so no on-chip transposes of big
tensors are needed:
  - host passes xT [DIM, B*S]; projections compute QT/KT/VT = W^T xT
    directly via PE matmuls (weights stationary). Wq is split into
    even-d / odd-d column planes so RoPE runs as full-width aligned
    DVE ops; planes are then merged into head-contiguous layout via
    SBUF->SBUF DMAs (compute engines require same-partition operands,
    DMAs don't). 1/sqrt(hd) is folded into Wq on the host.
  - scores are built as S^T[k, q] tiles; alibi (host-transposed, causal
    mask folded in as -1e9) is accumulated into the same PSUM tile with
    an identity matmul; exp runs on the scalar engine (no max-subtraction
    needed: scores are O(10)); PV contracts k with V' = [V | 1] stationary
    so row 64 of the PSUM accumulator is the softmax denominator.
  - out^T[d, q] tiles are exactly the lhsT the Wo projection needs.

Matmuls run in float32r (full-rate fp32 PE mode).
"""

import os

import numpy as np
import ml_dtypes

BF16 = ml_dtypes.bfloat16
B = 2
S = 2048
DIM = 2048
N_HEADS = 32
N_KV_HEADS = 8
HEAD_DIM = 64
HL = 4            # local q heads per core
N_CORES = 8
BS = B * S        # 4096
KT = DIM // 128   # 16 contraction tiles for projections
MS = 512          # m-strip width for projections
NSTRIP = BS // MS  # 8
QS = 512          # q-strip width in attention
ROPE_THETA = 10000.0
NEG = -1e9

_compiled = None


def _build_nc(iters=1):
    import concourse.bass as bass
    import concourse.tile as tile
    from concourse import bacc, mybir
    from concourse.masks import make_identity

    f32 = mybir.dt.float32
    bf16 = mybir.dt.bfloat16
    SUB = mybir.AluOpType.subtract
    ADD = mybir.AluOpType.add

    nc = bacc.Bacc("TRN2", target_bir_lowering=False, debug=False,
                   num_devices=N_CORES)

    xT = nc.dram_tensor("xT", [DIM, BS], bf16, kind="ExternalInput").ap()
    wqe = nc.dram_tensor("wqe", [DIM, 128], bf16, kind="ExternalInput").ap()
    wqo = nc.dram_tensor("wqo", [DIM, 128], bf16, kind="ExternalInput").ap()
    wkv = nc.dram_tensor("wkv", [DIM, 128], bf16, kind="ExternalInput").ap()
    wo = nc.dram_tensor("wo", [HL * HEAD_DIM, DIM], bf16, kind="ExternalInput").ap()
    alibiT = nc.dram_tensor("alibiT", [HL, S, S], bf16, kind="ExternalInput").ap()
    c128 = nc.dram_tensor("c128", [128, S], f32, kind="ExternalInput").ap()
    s128 = nc.dram_tensor("s128", [128, S], f32, kind="ExternalInput").ap()
    out = nc.dram_tensor("out", [BS, DIM], bf16, kind="ExternalOutput").ap()

    with tile.TileContext(nc) as tc, \
         tc.tile_pool(name="persist", bufs=1) as pers:
        # ---- persistent SBUF ----
        wqe_sb = pers.tile([128, KT, 128], bf16, name="wqe_sb")
        wqo_sb = pers.tile([128, KT, 128], bf16, name="wqo_sb")
        wkv_sb = pers.tile([128, KT, 128], bf16, name="wkv_sb")
        wo_sb = pers.tile([128, 2, DIM], bf16, name="wo_sb")
        c128_sb = pers.tile([128, S], f32, name="c128_sb")
        s128_sb = pers.tile([128, S], f32, name="s128_sb")
        ident = pers.tile([128, 128], bf16, name="ident")
        ones65 = pers.tile([65, 64], f32, name="ones65")
        qt0 = pers.tile([128, BS], bf16, name="qt0")   # heads 0,1 (rope'd QT)
        qt1 = pers.tile([128, BS], bf16, name="qt1")   # heads 2,3
        kt_sb = pers.tile([128, BS], bf16, name="kt_sb")  # KT dup'd both halves
        v_sb = pers.tile([128, 32, 65], bf16, name="v_sb")   # V' blocks [V|1]
        ot0 = pers.tile([128, BS], bf16, name="ot0")   # heads 0,1 outT
        ot1 = pers.tile([128, BS], bf16, name="ot1")

        nc.sync.dma_start(wqe_sb[:], wqe.rearrange("(t p) n -> p t n", p=128))
        nc.sync.dma_start(wqo_sb[:], wqo.rearrange("(t p) n -> p t n", p=128))
        nc.sync.dma_start(wkv_sb[:], wkv.rearrange("(t p) n -> p t n", p=128))
        nc.sync.dma_start(wo_sb[:], wo.rearrange("(t p) n -> p t n", p=128))
        nc.sync.dma_start(c128_sb[:], c128[:])
        nc.sync.dma_start(s128_sb[:], s128[:])
        make_identity(nc, ident[:])
        nc.vector.memset(ones65[:], 1.0)
        nc.vector.memset(v_sb[:, :, 64:65], 1.0)

        for _ in range(iters):
            # ================= phase 1: projections =================
            with tc.tile_pool(name="proj_x", bufs=2) as xp, \
                 tc.tile_pool(name="proj_ps", bufs=2, space="PSUM") as pp, \
                 tc.tile_pool(name="proj_tmp", bufs=8) as tp:
                for st in range(NSTRIP):
                    m0 = st * MS
                    s0 = m0 % S
                    qpe = pp.tile([128, MS], f32, tag="qpe", name="qpe")
                    qpo = pp.tile([128, MS], f32, tag="qpo", name="qpo")
                    kvp = pp.tile([128, MS], f32, tag="kvp", name="kvp")
                    for tq in range(4):   # quarters of the k loop
                        xt = xp.tile([128, 4, MS], bf16, tag="xt", name="xt")
                        nc.sync.dma_start(
                            xt[:],
                            xT[tq * 512:(tq + 1) * 512, m0:m0 + MS]
                            .rearrange("(t p) m -> p t m", p=128))
                        for t4 in range(4):
                            t = tq * 4 + t4
                            first = t == 0
                            last = t == KT - 1
                            for wsb, ps in ((wqe_sb, qpe), (wqo_sb, qpo),
                                            (wkv_sb, kvp)):
                                nc.tensor.matmul(
                                    ps[:], wsb[:, t, :],
                                    xt[:, t4, :],
                                    start=first, stop=last)
                    # ---- RoPE Q: full-width plane ops (all 4 heads) ----
                    qe_r = tp.tile([128, MS], bf16, tag="tmp", name="qe_r")
                    qo_r = tp.tile([128, MS], bf16, tag="tmp", name="qo_r")
                    t1 = tp.tile([128, MS], f32, tag="tmp", name="t1")
                    t2 = tp.tile([128, MS], f32, tag="tmp", name="t2")
                    cS = c128_sb[:, s0:s0 + MS]
                    sS = s128_sb[:, s0:s0 + MS]
                    nc.vector.tensor_mul(t1[:], qpe[:], cS)
                    nc.vector.tensor_mul(t2[:], qpo[:], sS)
                    nc.vector.tensor_tensor(qe_r[:], t1[:], t2[:], op=SUB)
                    t3 = tp.tile([128, MS], f32, tag="tmp", name="t3")
                    t4_ = tp.tile([128, MS], f32, tag="tmp", name="t4_")
                    nc.vector.tensor_mul(t3[:], qpe[:], sS)
                    nc.vector.tensor_mul(t4_[:], qpo[:], cS)
                    nc.vector.tensor_tensor(qo_r[:], t3[:], t4_[:], op=ADD)
                    # merge planes -> head-contiguous qt banks (DMA moves
                    # partitions; compute engines can't)
                    for h in range(HL):
                        qtb = (qt0, qt1)[h // 2]
                        hb = (h % 2) * 64
                        nc.sync.dma_start(
                            qtb[hb:hb + 32, m0:m0 + MS],
                            qe_r[h * 32:(h + 1) * 32, :])
                        nc.sync.dma_start(
                            qtb[hb + 32:hb + 64, m0:m0 + MS],
                            qo_r[h * 32:(h + 1) * 32, :])
                    # ---- RoPE K (rows 0-63 of kvp) ----
                    km = tp.tile([64, MS], f32, tag="tmp", name="km")
                    ka = tp.tile([32, MS], f32, tag="tmp", name="ka")
                    ko2 = tp.tile([32, MS], bf16, tag="tmp", name="ko2")
                    nc.vector.tensor_mul(km[0:32, :], kvp[0:32, :],
                                         c128_sb[0:32, s0:s0 + MS])
                    nc.vector.tensor_mul(km[32:64, :], kvp[32:64, :],
                                         s128_sb[32:64, s0:s0 + MS])
                    nc.sync.dma_start(ka[:], km[32:64, :])
                    nc.vector.tensor_tensor(kt_sb[0:32, m0:m0 + MS],
                                            km[0:32, :], ka[:], op=SUB)
                    km2 = tp.tile([64, MS], f32, tag="tmp", name="km2")
                    ka2 = tp.tile([32, MS], f32, tag="tmp", name="ka2")
                    nc.vector.tensor_mul(km2[0:32, :], kvp[0:32, :],
                                         s128_sb[0:32, s0:s0 + MS])
                    nc.vector.tensor_mul(km2[32:64, :], kvp[32:64, :],
                                         c128_sb[32:64, s0:s0 + MS])
                    nc.sync.dma_start(ka2[:], km2[32:64, :])
                    nc.vector.tensor_tensor(ko2[:], km2[0:32, :], ka2[:], op=ADD)
                    nc.sync.dma_start(kt_sb[32:64, m0:m0 + MS], ko2[:])
                    # duplicate rope'd KT into partitions 64-127 (matmul
                    # needs lhsT and rhs at the same base partition)
                    nc.sync.dma_start(kt_sb[64:128, m0:m0 + MS],
                                      kt_sb[0:64, m0:m0 + MS])
                    # ---- V (rows 64-127 of kvp): transpose to [k, d] ----
                    vts = tp.tile([128, MS], bf16, tag="tmp", name="vts")
                    nc.any.tensor_copy(vts[64:128, :], kvp[64:128, :])
                    for c in range(4):
                        vtp = pp.tile([128, 64], bf16, tag="vtp", name="vtp")
                        nc.tensor.transpose(vtp[:],
                                            vts[64:128, c * 128:(c + 1) * 128],
                                            ident[64:128, 64:128])
                        nc.any.tensor_copy(v_sb[:, st * 4 + c, 0:64], vtp[:])

            # ================= phase 2: attention =================
            with tc.tile_pool(name="att_al", bufs=4) as alp, \
                 tc.tile_pool(name="att_es", bufs=4) as esp, \
                 tc.tile_pool(name="att_st", bufs=3, space="PSUM") as stp, \
                 tc.tile_pool(name="att_ov", bufs=2, space="PSUM") as ovp, \
                 tc.tile_pool(name="att_rb", bufs=1, space="PSUM") as rbp, \
                 tc.tile_pool(name="att_sm", bufs=2) as smp:
                for h in range(HL):
                    qtb = (qt0, qt1)[h // 2]
                    otb = (ot0, ot1)[h // 2]
                    hb = (h % 2) * 64
                    for qs in range(S // QS):
                        q0 = qs * QS
                        nk = 4 * (qs + 1)
                        ov = [ovp.tile([65, QS], f32, tag=f"ov{b}", name=f"ov{b}")
                              for b in range(B)]
                        for jk in range(nk):
                            k0 = jk * 128
                            al = alp.tile([128, QS], bf16, tag="al", name="al")
                            nc.sync.dma_start(
                                al[:], alibiT[h, k0:k0 + 128, q0:q0 + QS])
                            for b in range(B):
                                stt = stp.tile([128, QS], f32, tag="st", name="st")
                                nc.tensor.matmul(
                                    stt[:],
                                    kt_sb[hb:hb + 64,
                                          b * S + k0:b * S + k0 + 128],
                                    qtb[hb:hb + 64, b * S + q0:b * S + q0 + QS],
                                    start=True, stop=True)
                                esr = esp.tile([128, QS], bf16, tag="esr",
                                               name="esr")
                                nc.scalar.activation(
                                    esr[:], stt[:],
                                    mybir.ActivationFunctionType.Exp)
                                es = esp.tile([128, QS], bf16, tag="es", name="es")
                                nc.vector.tensor_mul(es[:], esr[:], al[:])
                                nc.tensor.matmul(
                                    ov[b][:],
                                    v_sb[:, b * 16 + jk, :],
                                    es[:],
                                    start=(jk == 0), stop=(jk == nk - 1))
                        for b in range(B):
                            rc = smp.tile([65, QS], f32, tag="rc", name="rc")
                            nc.vector.reciprocal(rc[64:65, :], ov[b][64:65, :])
                            rb = rbp.tile([64, QS], f32, tag="rb", name="rb")
                            nc.tensor.matmul(
                                rb[:], ones65[64:65, :],
                                rc[64:65, :],
                                start=True, stop=True)
                            rbs = smp.tile([64, QS], f32, tag="rbs",
                                           name="rbs")
                            nc.any.tensor_copy(rbs[:], rb[:])
                            dst = b * S + q0
                            if hb == 0:
                                nc.vector.tensor_mul(
                                    otb[0:64, dst:dst + QS],
                                    ov[b][0:64, :], rbs[:])
                            else:
                                om = smp.tile([64, QS], bf16, tag="om", name="om")
                                nc.vector.tensor_mul(om[:], ov[b][0:64, :],
                                                     rbs[:])
                                nc.sync.dma_start(otb[64:128, dst:dst + QS],
                                                  om[:])

            # ================= phase 3: output projection =================
            with tc.tile_pool(name="out_ps", bufs=3, space="PSUM") as opp, \
                 tc.tile_pool(name="out_sb", bufs=3) as osp:
                for mt in range(BS // 128):
                    for ns in range(DIM // 512):
                        op = opp.tile([128, 512], f32, tag="op", name="op")
                        nc.tensor.matmul(
                            op[:], ot0[:, mt * 128:(mt + 1) * 128],
                            wo_sb[:, 0, ns * 512:(ns + 1) * 512],
                            start=True, stop=False)
                        nc.tensor.matmul(
                            op[:], ot1[:, mt * 128:(mt + 1) * 128],
                            wo_sb[:, 1, ns * 512:(ns + 1) * 512],
                            start=False, stop=True)
                        ob = osp.tile([128, 512], bf16, tag="ob", name="ob")
                        if (mt + ns) % 3 == 0:
                            nc.vector.tensor_copy(ob[:], op[:])
                        else:
                            nc.scalar.copy(ob[:], op[:])
                        nc.sync.dma_start(
                            out[mt * 128:(mt + 1) * 128,
                                ns * 512:(ns + 1) * 512], ob[:])

    nc.compile()
    return nc


def _rope_tables():
    inv = 1.0 / (ROPE_THETA ** (np.arange(0, HEAD_DIM, 2, dtype=np.float64)
                                / HEAD_DIM))
    ang = np.arange(S, dtype=np.float64)[None, :] * inv[:, None]   # [32, S]
    cos = np.cos(ang).astype(np.float32)
    sin = np.sin(ang).astype(np.float32)
    c128 = np.ascontiguousarray(np.tile(cos, (4, 1)))
    s128 = np.ascontiguousarray(np.tile(sin, (4, 1)))
    return c128, s128


def shard_inputs(x, alibi_bias, Wq, Wk, Wv, Wo):
    x = np.asarray(x, dtype=np.float32)
    alibi_bias = np.asarray(alibi_bias, dtype=np.float32)
    Wq = np.asarray(Wq, dtype=np.float32)
    Wk = np.asarray(Wk, dtype=np.float32)
    Wv = np.asarray(Wv, dtype=np.float32)
    Wo = np.asarray(Wo, dtype=np.float32)

    xT_bf = np.ascontiguousarray(x.reshape(BS, DIM).T).astype(BF16)
    c128, s128 = _rope_tables()
    ev = np.arange(0, HEAD_DIM, 2)
    od = np.arange(1, HEAD_DIM, 2)
    perm = np.concatenate([ev, od])
    # causal mask in transposed [k, q] layout: invalid where k > q
    ktri = np.tril(np.full((S, S), NEG, dtype=np.float32), k=-1)

    in_maps = []
    scale = np.float32(1.0 / np.sqrt(HEAD_DIM))
    for g in range(N_CORES):
        hs = np.arange(4 * g, 4 * g + 4)
        e_cols = np.concatenate([h * HEAD_DIM + ev for h in hs])
        o_cols = np.concatenate([h * HEAD_DIM + od for h in hs])
        wqe_g = np.ascontiguousarray(Wq[:, e_cols]) * scale
        wqo_g = np.ascontiguousarray(Wq[:, o_cols]) * scale
        wk_g = Wk[:, g * HEAD_DIM + perm]
        wv_g = Wv[:, g * HEAD_DIM:(g + 1) * HEAD_DIM]
        wkv_g = np.ascontiguousarray(np.concatenate([wk_g, wv_g], axis=1))
        wo_g = np.ascontiguousarray(Wo[4 * g * HEAD_DIM:(4 * g + 4) * HEAD_DIM])
        at = np.exp(alibi_bias[0, hs].transpose(0, 2, 1) + ktri[None])
        in_maps.append({
            "xT": xT_bf,
            "wqe": wqe_g.astype(BF16),
            "wqo": wqo_g.astype(BF16),
            "wkv": wkv_g.astype(BF16),
            "wo": wo_g.astype(BF16),
            "alibiT": np.ascontiguousarray(at).astype(BF16),
            "c128": c128, "s128": s128,
        })
    return in_maps


def get_compiled(iters=None):
    global _compiled
    if iters is None:
        iters = int(os.environ.get("BASS_ITERS", "1"))
    if _compiled is None or _compiled[0] != iters:
        _compiled = (iters, _build_nc(iters))
    return _compiled[1]


def kernel(**inputs):
    from concourse.bass_utils import run_bass_kernel_spmd

    nc = get_compiled()
    in_maps = shard_inputs(**inputs)
    res = run_bass_kernel_spmd(nc, in_maps, core_ids=list(range(N_CORES)))
    total = np.zeros((BS, DIM), dtype=np.float32)
    for r in res.results:
        total += np.asarray(r["out"], dtype=np.float32)
    return total.reshape(B, S, DIM)
